# revision 13
# baseline (speedup 1.0000x reference)
"""Trainium2 Bass kernel for the BreakthroughSNN model.

Strategy (per sharding hint): the SSM/attention recurrence couples all B*S
positions each step and per-step cross-core collectives have a ~60us floor,
so the recurrent stack is replicated on all 8 cores; the vocab-sized output
projection (the memory-bound part) is tensor-parallel over vocab (4000
columns per core).

On-device layout is feature-major ("transposed"): activations are stored as
[features(partitions), positions(free)] so matmuls contract over partitions
and softmax normalization sums run through the PE via an appended
ones-column on V.
"""

import sys
import numpy as np

for _p in ("/opt/trn_rl_repo",):
    if _p not in sys.path:
        sys.path.append(_p)

import ml_dtypes
import concourse.bass as bass
import concourse.mybir as mybir
import concourse.tile as tile
from concourse import bacc
from concourse.bass_utils import run_bass_kernel_spmd
from concourse.masks import make_identity

bf16 = ml_dtypes.bfloat16
DT = mybir.dt
AF = mybir.ActivationFunctionType
OP = mybir.AluOpType

# model dims
B, S, T = 4, 256, 8
DM, DS, NH, L = 256, 128, 4, 2
DH = DS // NH
V = 32000
R = B * S                      # 1024 positions
NCORES = 8
VS = V // NCORES               # 4000 vocab per core
TAU, BASE_THR, ADAPT, TARGET = 2.0, 1.0, 0.1, 0.02
DECAY = float(np.exp(-1.0 / TAU))
SCALE = 1.0 / float(np.sqrt(DH))

_CACHE = {}


def _build():
    if "nc" in _CACHE:
        return _CACHE["nc"]

    nc = bacc.Bacc("TRN2", target_bir_lowering=False, debug=False,
                   num_devices=NCORES)

    f32, bf, i32 = DT.float32, DT.bfloat16, DT.int32

    # ---------------- DRAM parameters ----------------
    idx_d = nc.dram_tensor("idx", [R, 1], i32, kind="ExternalInput").ap()
    emb_d = nc.dram_tensor("emb", [V, DM], f32, kind="ExternalInput").ap()
    AT_d, WqT_d, WkT_d, WvT_d, WoT_d, CT_d = [], [], [], [], [], []
    bq_d, bk_d, bv_d, bo_d, boR_d = [], [], [], [], []
    for l in range(L):
        AT_d.append(nc.dram_tensor(f"AT{l}", [DS, DS], bf, kind="ExternalInput").ap())
        WqT_d.append(nc.dram_tensor(f"WqT{l}", [DS, DS], bf, kind="ExternalInput").ap())
        WkT_d.append(nc.dram_tensor(f"WkT{l}", [128, 256], bf, kind="ExternalInput").ap())
        WvT_d.append(nc.dram_tensor(f"WvT{l}", [128, 256], bf, kind="ExternalInput").ap())
        WoT_d.append(nc.dram_tensor(f"WoT{l}", [DS, DS], bf, kind="ExternalInput").ap())
        CT_d.append(nc.dram_tensor(f"CT{l}", [DS, DM], bf, kind="ExternalInput").ap())
        bq_d.append(nc.dram_tensor(f"bq{l}", [DS, 1], f32, kind="ExternalInput").ap())
        bk_d.append(nc.dram_tensor(f"bk{l}", [DS, 1], f32, kind="ExternalInput").ap())
        bv_d.append(nc.dram_tensor(f"bv{l}", [1, DS], bf, kind="ExternalInput").ap())
        bo_d.append(nc.dram_tensor(f"bo{l}", [DS, 1], f32, kind="ExternalInput").ap())
        boR_d.append(nc.dram_tensor(f"boR{l}", [1, DS], bf, kind="ExternalInput").ap())
    VSP = 4096
    WoutT_d = nc.dram_tensor("WoutT", [128, 2 * VSP], DT.float32r, kind="ExternalInput").ap()
    bout_d = nc.dram_tensor("bout", [1, VSP], DT.float32r, kind="ExternalInput").ap()
    onesr_d = nc.dram_tensor("onesr", [1, 128], DT.float32r, kind="ExternalInput").ap()

    logits_d = nc.dram_tensor("logits", [R, VSP], f32, kind="ExternalOutput").ap()
    ti_dbg_d = nc.dram_tensor("ti_dbg", [128, 2 * R], f32, kind="ExternalOutput").ap()

    # internal DRAM bounce buffers (for partition broadcasts)
    rc_b = nc.dram_tensor("rc_b", [NH, 512], f32).ap()
    th_b = nc.dram_tensor("th_b", [2, 1], f32).ap()

    with tile.TileContext(nc) as tc:
        with tc.tile_pool(name="state", bufs=1) as st, \
             tc.tile_pool(name="wk", bufs=1) as wk, \
             tc.tile_pool(name="sc", bufs=3) as sc, \
             tc.tile_pool(name="pr", bufs=8) as pr, \
             tc.tile_pool(name="psB", bufs=4, space="PSUM") as psB, \
             tc.tile_pool(name="psS", bufs=4, space="PSUM") as psS:

            # ---------------- persistent SBUF ----------------
            ident = st.tile([128, 128], f32)
            make_identity(nc, ident)
            identb = st.tile([128, 128], bf)
            make_identity(nc, identb)
            ones512 = st.tile([1, 512], bf)
            nc.vector.memset(ones512[:], 1.0)
            ones_r = st.tile([1, 128], bf)       # ones row, lhsT for bias mms
            nc.vector.memset(ones_r[:], 1.0)
            ones_rf = st.tile([1, 128], DT.float32r)
            nc.sync.dma_start(out=ones_rf[:], in_=onesr_d[:])
            ones_c = st.tile([128, 1], f32)      # ones col, partition reduce
            nc.vector.memset(ones_c[:], 1.0)

            x0T = st.tile([128, 2 * R], bf)      # tok transposed, 2 dm-chunks
            k1T = st.tile([128, R], bf)
            k2T = st.tile([128, R], bf)
            k2Tb = st.tile([128, R], bf)
            v1p = st.tile([128, 8 * 132], bf)    # [kpos, rt*(4 heads*(32v+1))]
            v2p = st.tile([128, 8 * 132], bf)
            v2pb = st.tile([128, 8 * 132], bf)
            hT = st.tile([128, R], bf)
            svT = st.tile([128, R], bf)          # stored pre-decayed
            ovT = st.tile([128, 2 * R], bf)      # 2 dm-chunks, pre-decayed
            thr_s = st.tile([128, 1], f32)
            thr_o = st.tile([128, 1], f32)
            os1 = st.tile([128, T * 2 * R], bf)  # layer-1 spikes, all steps
            tiT = st.tile([128, 2 * R], f32)
            attn = st.tile([128, R], bf)
            q_sb = st.tile([128, R], bf)
            keep = st.tile([128, R], f32)
            os2 = st.tile([128, 2 * R], bf)
            scnt = st.tile([128, 8], f32)        # spike rowsums per half/block
            thrw = st.tile([1, 4], f32)          # small thr scratch

            # weights
            ATs, WqTs, WkTs, WvTs, WoTs, CTs = [], [], [], [], [], []
            bqs, bks, bvs, bos, boRs = [], [], [], [], []
            for l in range(L):
                ATs.append(wk.tile([DS, DS], bf, tag=f"AT{l}", name=f"ATs{l}"))
                WqTs.append(wk.tile([DS, DS], bf, tag=f"Wq{l}", name=f"WqTs{l}"))
                WkTs.append(wk.tile([128, 256], bf, tag=f"Wk{l}", name=f"WkTs{l}"))
                WvTs.append(wk.tile([128, 256], bf, tag=f"Wv{l}", name=f"WvTs{l}"))
                WoTs.append(wk.tile([DS, DS], bf, tag=f"Wo{l}", name=f"WoTs{l}"))
                CTs.append(wk.tile([DS, DM], bf, tag=f"CT{l}", name=f"CTs{l}"))
                bqs.append(wk.tile([DS, 1], f32, tag=f"bq{l}", name=f"bqs{l}"))
                bks.append(wk.tile([DS, 1], f32, tag=f"bk{l}", name=f"bks{l}"))
                bvs.append(wk.tile([1, DS], bf, tag=f"bv{l}", name=f"bvs{l}"))
                bos.append(wk.tile([DS, 1], f32, tag=f"bo{l}", name=f"bos{l}"))
                boRs.append(wk.tile([1, DS], bf, tag=f"boR{l}", name=f"boRs{l}"))
                nc.sync.dma_start(out=boRs[l][:], in_=boR_d[l][:])
                nc.sync.dma_start(out=ATs[l][:], in_=AT_d[l][:])
                nc.sync.dma_start(out=WqTs[l][:], in_=WqT_d[l][:])
                nc.sync.dma_start(out=WkTs[l][:], in_=WkT_d[l][:])
                nc.sync.dma_start(out=WvTs[l][:], in_=WvT_d[l][:])
                nc.sync.dma_start(out=WoTs[l][:], in_=WoT_d[l][:])
                nc.sync.dma_start(out=CTs[l][:], in_=CT_d[l][:])
                nc.sync.dma_start(out=bqs[l][:], in_=bq_d[l][:])
                nc.sync.dma_start(out=bks[l][:], in_=bk_d[l][:])
                nc.sync.dma_start(out=bvs[l][:], in_=bv_d[l][:])
                nc.sync.dma_start(out=bos[l][:], in_=bo_d[l][:])
            Wout_sb = wk.tile([128, 2 * VSP], DT.float32r, tag="wout")
            nc.sync.dma_start(out=Wout_sb[:], in_=WoutT_d[:])
            bout_sb = wk.tile([1, VSP], DT.float32r, tag="bout")
            nc.sync.dma_start(out=bout_sb[:], in_=bout_d[:])

            # ---------------- embedding gather + transpose ----------------
            for rt in range(8):
                idx_sb = sc.tile([128, 1], i32, tag="idx")
                nc.sync.dma_start(out=idx_sb[:], in_=idx_d[128 * rt:128 * (rt + 1), :])
                tok_sb = sc.tile([128, DM], f32, tag="tok")
                nc.gpsimd.indirect_dma_start(
                    out=tok_sb[:], out_offset=None, in_=emb_d[:],
                    in_offset=bass.IndirectOffsetOnAxis(ap=idx_sb[:, :1], axis=0))
                for mc in range(2):
                    ps_t = psB.tile([128, 128], f32, tag="work")
                    nc.tensor.transpose(out=ps_t[:], in_=tok_sb[:, 128 * mc:128 * (mc + 1)],
                                        identity=ident[:])
                    nc.vector.tensor_copy(
                        out=x0T[:, R * mc + 128 * rt: R * mc + 128 * (rt + 1)],
                        in_=ps_t[:])

            # ---------------- init state ----------------
            nc.gpsimd.memset(hT[:], 0.0)
            nc.gpsimd.memset(svT[:], 0.0)
            nc.gpsimd.memset(ovT[:], 0.0)
            nc.gpsimd.memset(tiT[:], 0.0)
            nc.vector.memset(thr_s[:], BASE_THR)
            nc.vector.memset(thr_o[:], BASE_THR)
            # ones columns of v_pos tiles
            for vp in (v1p, v2p, v2pb):
                vv = vp[:].rearrange("p (r h c) -> p r h c", r=8, h=NH, c=33)
                nc.gpsimd.memset(vv[:, :, :, 32:33], 1.0)

            def kv_for(l, xT_ap_fn, kdst, vdst):
                """xT_ap_fn(kc) -> [128, R] feature-major input chunk."""
                for nh in range(2):
                    ps_k = psB.tile([128, 512], f32, tag="work")
                    for kc in range(2):
                        nc.tensor.matmul(ps_k[:],
                                         lhsT=WkTs[l][:, 128 * kc:128 * (kc + 1)],
                                         rhs=xT_ap_fn(kc)[:, 512 * nh:512 * (nh + 1)],
                                         start=(kc == 0), stop=(kc == 1))
                    nc.vector.tensor_scalar(out=kdst[:, 512 * nh:512 * (nh + 1)],
                                            in0=ps_k[:], scalar1=bks[l][:, :1],
                                            scalar2=None, op0=OP.add)
                vv = vdst[:].rearrange("p (r h c) -> p r h c", r=8, h=NH, c=33)
                for rt in range(8):
                    ps_v = psB.tile([128, 128], f32, tag="work")
                    for kc in range(2):
                        nc.tensor.matmul(ps_v[:],
                                         lhsT=xT_ap_fn(kc)[:, 128 * rt:128 * (rt + 1)],
                                         rhs=WvTs[l][:, 128 * kc:128 * (kc + 1)],
                                         start=(kc == 0), stop=False)
                    nc.tensor.matmul(ps_v[:], lhsT=ones_r[:], rhs=bvs[l][:],
                                     start=False, stop=True)
                    src = ps_v[:].rearrange("p (h c) -> p h c", h=NH, c=32)
                    nc.vector.tensor_copy(out=vv[:, rt, :, 0:32], in_=src)

            def lif(ps_upd, sv_ap, thr_ap, spike_dst, scnt_col, kslice):
                """Adaptive-LIF on a [128, 512] half-block.
                ps_upd already holds upd + sv(pre-decayed) (+ bias)."""
                nc.vector.tensor_scalar(out=spike_dst, in0=ps_upd[:],
                                        scalar1=thr_ap, scalar2=None, op0=OP.is_ge)
                nc.vector.tensor_scalar(out=keep[:, kslice], in0=ps_upd[:],
                                        scalar1=thr_ap, scalar2=DECAY,
                                        op0=OP.is_lt, op1=OP.mult)
                nc.vector.tensor_tensor(out=sv_ap, in0=ps_upd[:], in1=keep[:, kslice],
                                        op=OP.mult)
                nc.vector.tensor_reduce(scnt[:, scnt_col:scnt_col + 1], spike_dst,
                                        axis=mybir.AxisListType.X, op=OP.add)

            def thr_update(thr_tile, cols, denom, th_row):
                """thr += ADAPT*(mean-TARGET); thr = max(thr, 0.5); rebroadcast."""
                ps_tot = psB.tile([1, 1], f32, tag="work")
                for i, c in enumerate(cols):
                    nc.tensor.matmul(ps_tot[:], lhsT=ones_c[:],
                                     rhs=scnt[:, c:c + 1],
                                     start=(i == 0), stop=(i == len(cols) - 1))
                d = thrw[:, th_row:th_row + 1]
                nc.vector.tensor_scalar(out=d, in0=ps_tot[:],
                                        scalar1=ADAPT / denom,
                                        scalar2=-ADAPT * TARGET,
                                        op0=OP.mult, op1=OP.add)
                nc.vector.tensor_tensor(out=d, in0=d, in1=thr_tile[0:1, :], op=OP.add)
                nc.vector.tensor_scalar(out=d, in0=d, scalar1=0.5, scalar2=None,
                                        op0=OP.max)
                nc.sync.dma_start(out=th_b[th_row:th_row + 1, :], in_=d)
                nc.sync.dma_start(out=thr_tile[:],
                                  in_=th_b[th_row:th_row + 1, :].to_broadcast((128, 1)))

            # ---------------- the recurrence ----------------
            kv_for(0, lambda kc: x0T[:, R * kc:R * (kc + 1)], k1T, v1p)

            for l in range(L):
                for t in range(T):
                    if l == 0:
                        k_sb, v_sb = k1T, v1p
                    else:
                        k_sb = k2T if t % 2 == 0 else k2Tb
                        v_sb = v2p if t % 2 == 0 else v2pb
                        xfn = lambda kc, t=t: os1[:, 2 * R * t + R * kc:2 * R * t + R * (kc + 1)]
                        kv_for(1, xfn, k_sb, v_sb)

                    for qh in range(2):
                        Q = slice(512 * qh, 512 * (qh + 1))

                        # qT = Wq @ hT + bq  (bias applied on ScalarE)
                        ps_q = psB.tile([128, 512], f32, tag="work")
                        nc.tensor.matmul(ps_q[:], lhsT=WqTs[l][:], rhs=hT[:, Q],
                                         start=True, stop=True)
                        nc.scalar.add(out=q_sb[:, Q], in_=ps_q[:], add=bqs[l][:, :1])

                        # attention: scoresT -> exp -> att (+sums via ones col)
                        pa = [psS.tile([128, 512], f32, tag="pa", name=f"pa{p}")
                              for p in range(2)]
                        for kt in range(8):
                            for p in range(2):
                                for j in range(2):
                                    h = 2 * p + j
                                    ps_sc = psB.tile([128, 512], f32, tag="work",
                                                     name=f"ps_sc{h}")
                                    nc.tensor.matmul(
                                        ps_sc[:],
                                        lhsT=k_sb[32 * h:32 * (h + 1), 128 * kt:128 * (kt + 1)],
                                        rhs=q_sb[32 * h:32 * (h + 1), Q],
                                        start=True, stop=True,
                                        tile_position=(32 * h, 0))
                                    probs = pr.tile([128, 512], bf, tag="probs")
                                    nc.scalar.activation(probs[:], ps_sc[:], AF.Exp,
                                                         bias=0.0, scale=SCALE)
                                    nc.tensor.matmul(
                                        pa[p][64 * j:64 * j + 33, :],
                                        lhsT=v_sb[:, 132 * kt + 33 * h:132 * kt + 33 * (h + 1)],
                                        rhs=probs[:],
                                        start=(kt == 0), stop=(kt == 7),
                                        tile_position=(0, 64 * j))
                        srows = sc.tile([97, 512], f32, tag="srows")
                        for h in range(NH):
                            p, j = h // 2, h % 2
                            nc.vector.tensor_copy(out=srows[32 * h:32 * h + 1, :],
                                                  in_=pa[p][64 * j + 32:64 * j + 33, :])
                        recip = sc.tile([97, 512], f32, tag="recip")
                        nc.vector.reciprocal_approx_fast(out=recip[:], in_=srows[:])
                        for h in range(NH):
                            nc.sync.dma_start(out=rc_b[h:h + 1, :],
                                              in_=recip[32 * h:32 * h + 1, :])
                        rb = sc.tile([128, 512], f32, tag="rb")
                        for h in range(NH):
                            nc.sync.dma_start(
                                out=rb[32 * h:32 * (h + 1), :],
                                in_=rc_b[h:h + 1, :].to_broadcast((32, 512)))
                        for h in range(NH):
                            p, j = h // 2, h % 2
                            nc.vector.tensor_tensor(
                                out=attn[32 * h:32 * (h + 1), Q],
                                in0=pa[p][64 * j:64 * j + 32, :],
                                in1=rb[32 * h:32 * (h + 1), :], op=OP.mult)

                        # upd = A@hT + Wo@attn + sv + bo  (all accumulated in PSUM)
                        ps_u = psB.tile([128, 512], f32, tag="work")
                        nc.tensor.matmul(ps_u[:], lhsT=ATs[l][:], rhs=hT[:, Q],
                                         start=True, stop=False)
                        nc.tensor.matmul(ps_u[:], lhsT=WoTs[l][:], rhs=attn[:, Q],
                                         start=False, stop=False)
                        nc.tensor.matmul(ps_u[:], lhsT=identb[:], rhs=svT[:, Q],
                                         start=False, stop=False)
                        nc.tensor.matmul(ps_u[:], lhsT=boRs[l][:], rhs=ones512[:],
                                         start=False, stop=True)
                        lif(ps_u, svT[:, Q], thr_s[:, :1], hT[:, Q], 4 * qh + 0, Q)

                        # out_pot = C @ h2T + ov ; second LIF
                        for mc in range(2):
                            ps_o = psB.tile([128, 512], f32, tag="work")
                            nc.tensor.matmul(ps_o[:], lhsT=CTs[l][:, 128 * mc:128 * (mc + 1)],
                                             rhs=hT[:, Q], start=True, stop=False)
                            nc.tensor.matmul(ps_o[:], lhsT=identb[:],
                                             rhs=ovT[:, R * mc + 512 * qh:R * mc + 512 * (qh + 1)],
                                             start=False, stop=True)
                            if l == 0:
                                sdst = os1[:, 2 * R * t + R * mc + 512 * qh:
                                           2 * R * t + R * mc + 512 * (qh + 1)]
                            else:
                                sdst = os2[:, R * mc + 512 * qh:R * mc + 512 * (qh + 1)]
                            lif(ps_o, ovT[:, R * mc + 512 * qh:R * mc + 512 * (qh + 1)],
                                thr_o[:, :1], sdst, 4 * qh + 1 + mc, Q)
                            if l == 1:
                                nc.vector.tensor_tensor(
                                    out=tiT[:, R * mc + 512 * qh:R * mc + 512 * (qh + 1)],
                                    in0=tiT[:, R * mc + 512 * qh:R * mc + 512 * (qh + 1)],
                                    in1=sdst, op=OP.add)

                    thr_update(thr_s, [0, 4], float(R * DS), 0)
                    thr_update(thr_o, [1, 2, 5, 6], float(R * DM), 1)

                if l == 0:
                    # reset state for layer 2
                    nc.gpsimd.memset(hT[:], 0.0)
                    nc.gpsimd.memset(svT[:], 0.0)
                    nc.gpsimd.memset(ovT[:], 0.0)
                    nc.vector.memset(thr_s[:], BASE_THR)
                    nc.vector.memset(thr_o[:], BASE_THR)

            # time integration (mean over T)
            tiP = st.tile([128, 2 * R], DT.float32r)
            nc.vector.tensor_scalar(out=tiP[:], in0=tiT[:], scalar1=1.0 / T,
                                    scalar2=None, op0=OP.mult)
            nc.sync.dma_start(out=ti_dbg_d[:], in_=tiP[:].bitcast(DT.float32))

            # ---------------- output projection (fp32r) ----------------
            ti_r = tiP[:]
            w_r = Wout_sb[:]
            b_r = bout_sb[:]
            ones_rr = ones_rf[:]
            nchunk = [(i * 512, 512) for i in range(VSP // 512)]
            for pt in range(8):
                for (c0, cw) in nchunk:
                    ps_l = psB.tile([128, 512], f32, tag="work")
                    nc.tensor.matmul(ps_l[:, :cw], lhsT=ones_rr[:, :128],
                                     rhs=b_r[:, c0:c0 + cw], start=True, stop=False)
                    for kc in range(2):
                        nc.tensor.matmul(
                            ps_l[:, :cw],
                            lhsT=ti_r[:, R * kc + 128 * pt:R * kc + 128 * (pt + 1)],
                            rhs=w_r[:, VSP * kc + c0:VSP * kc + c0 + cw],
                            start=False, stop=(kc == 1))
                    o_sb = sc.tile([128, 512], f32, tag="osb")
                    if (pt + (c0 // 512)) % 2 == 0:
                        nc.vector.tensor_copy(out=o_sb[:, :cw], in_=ps_l[:, :cw])
                    else:
                        nc.scalar.copy(out=o_sb[:, :cw], in_=ps_l[:, :cw])
                    nc.sync.dma_start(out=logits_d[128 * pt:128 * (pt + 1), c0:c0 + cw],
                                      in_=o_sb[:, :cw])

    nc.compile()
    _CACHE["nc"] = nc
    return nc


def kernel(input_ids, emb, A, C, Wq, bq, Wkv, bkv, Wo, bo, Wout, bout):
    nc = _build()

    input_ids = np.asarray(input_ids)
    emb = np.ascontiguousarray(np.asarray(emb, dtype=np.float32))
    A = np.asarray(A, dtype=np.float32)
    C = np.asarray(C, dtype=np.float32)
    Wq = np.asarray(Wq, dtype=np.float32)
    bq = np.asarray(bq, dtype=np.float32)
    Wkv = np.asarray(Wkv, dtype=np.float32)
    bkv = np.asarray(bkv, dtype=np.float32)
    Wo = np.asarray(Wo, dtype=np.float32)
    bo = np.asarray(bo, dtype=np.float32)
    Wout = np.asarray(Wout, dtype=np.float32)
    bout = np.asarray(bout, dtype=np.float32)

    idx = np.ascontiguousarray(input_ids.reshape(R, 1).astype(np.int32))

    base = {"idx": idx, "emb": emb}
    for l in range(L):
        Wk = Wkv[l][:DS]      # [128, 256]
        Wv = Wkv[l][DS:]      # [128, 256]
        WkT = Wk.T            # [256, 128]
        WvT = Wv.T
        base[f"AT{l}"] = np.ascontiguousarray(A[l].T).astype(bf16)
        base[f"WqT{l}"] = np.ascontiguousarray(Wq[l].T).astype(bf16)
        base[f"WkT{l}"] = np.ascontiguousarray(
            np.concatenate([WkT[:128], WkT[128:]], axis=1)).astype(bf16)
        base[f"WvT{l}"] = np.ascontiguousarray(
            np.concatenate([WvT[:128], WvT[128:]], axis=1)).astype(bf16)
        base[f"WoT{l}"] = np.ascontiguousarray(Wo[l].T).astype(bf16)
        base[f"CT{l}"] = np.ascontiguousarray(C[l].T).astype(bf16)
        base[f"bq{l}"] = np.ascontiguousarray(bq[l].reshape(DS, 1))
        base[f"bk{l}"] = np.ascontiguousarray(bkv[l][:DS].reshape(DS, 1))
        base[f"bv{l}"] = np.ascontiguousarray(bkv[l][DS:].reshape(1, DS)).astype(bf16)
        base[f"bo{l}"] = np.ascontiguousarray(bo[l].reshape(DS, 1))
        base[f"boR{l}"] = np.ascontiguousarray(bo[l].reshape(1, DS)).astype(bf16)

    in_maps = []
    for c in range(NCORES):
        m = dict(base)
        VSP = 4096
        WoT_c = np.zeros((256, VSP), np.float32)
        WoT_c[:, :VS] = Wout[c * VS:(c + 1) * VS].T
        m["WoutT"] = np.ascontiguousarray(
            np.concatenate([WoT_c[:128], WoT_c[128:]], axis=1))
        bo_c = np.zeros((1, VSP), np.float32)
        bo_c[0, :VS] = bout[c * VS:(c + 1) * VS]
        m["bout"] = bo_c
        m["onesr"] = np.ones((1, 128), np.float32)
        in_maps.append(m)

    res = run_bass_kernel_spmd(nc, in_maps, list(range(NCORES)))
    kernel.last_results = res

    logits = np.concatenate([res.results[c]["logits"][:, :VS]
                             for c in range(NCORES)], axis=1)
    return logits.reshape(B, S, V)


# revision 14
# speedup vs baseline: 1.2817x; 1.2817x over previous
"""Trainium2 Bass kernel for the BreakthroughSNN model.

Strategy (per sharding hint): the SSM/attention recurrence couples all B*S
positions each step and per-step cross-core collectives have a ~60us floor,
so the recurrent stack is replicated on all 8 cores; the vocab-sized output
projection (the memory-bound part) is tensor-parallel over vocab (4000
columns per core).

On-device layout is feature-major ("transposed"): activations are stored as
[features(partitions), positions(free)] so matmuls contract over partitions
and softmax normalization sums run through the PE via an appended
ones-column on V.
"""

import sys
import numpy as np

for _p in ("/opt/trn_rl_repo",):
    if _p not in sys.path:
        sys.path.append(_p)

import ml_dtypes
import concourse.bass as bass
import concourse.mybir as mybir
import concourse.tile as tile
from concourse import bacc
from concourse.bass_utils import run_bass_kernel_spmd
from concourse.masks import make_identity

bf16 = ml_dtypes.bfloat16
DT = mybir.dt
AF = mybir.ActivationFunctionType
OP = mybir.AluOpType

# model dims
B, S, T = 4, 256, 8
DM, DS, NH, L = 256, 128, 4, 2
DH = DS // NH
V = 32000
R = B * S                      # 1024 positions
NCORES = 8
VS = V // NCORES               # 4000 vocab per core
TAU, BASE_THR, ADAPT, TARGET = 2.0, 1.0, 0.1, 0.02
DECAY = float(np.exp(-1.0 / TAU))
SCALE = 1.0 / float(np.sqrt(DH))

_CACHE = {}


def _build():
    if "nc" in _CACHE:
        return _CACHE["nc"]

    nc = bacc.Bacc("TRN2", target_bir_lowering=False, debug=False,
                   num_devices=NCORES)

    f32, bf, i32 = DT.float32, DT.bfloat16, DT.int32

    # ---------------- DRAM parameters ----------------
    idx_d = nc.dram_tensor("idx", [R, 1], i32, kind="ExternalInput").ap()
    emb_d = nc.dram_tensor("emb", [V, DM], f32, kind="ExternalInput").ap()
    AT_d, WqT_d, WkT_d, WvT_d, WoT_d, CT_d = [], [], [], [], [], []
    bq_d, bk_d, bv_d, bo_d, boR_d = [], [], [], [], []
    for l in range(L):
        AT_d.append(nc.dram_tensor(f"AT{l}", [DS, DS], bf, kind="ExternalInput").ap())
        WqT_d.append(nc.dram_tensor(f"WqT{l}", [DS, DS], bf, kind="ExternalInput").ap())
        WkT_d.append(nc.dram_tensor(f"WkT{l}", [128, 256], bf, kind="ExternalInput").ap())
        WvT_d.append(nc.dram_tensor(f"WvT{l}", [128, 256], bf, kind="ExternalInput").ap())
        WoT_d.append(nc.dram_tensor(f"WoT{l}", [DS, DS], bf, kind="ExternalInput").ap())
        CT_d.append(nc.dram_tensor(f"CT{l}", [DS, DM], bf, kind="ExternalInput").ap())
        bq_d.append(nc.dram_tensor(f"bq{l}", [DS, 1], f32, kind="ExternalInput").ap())
        bk_d.append(nc.dram_tensor(f"bk{l}", [DS, 1], f32, kind="ExternalInput").ap())
        bv_d.append(nc.dram_tensor(f"bv{l}", [1, DS], bf, kind="ExternalInput").ap())
        bo_d.append(nc.dram_tensor(f"bo{l}", [DS, 1], f32, kind="ExternalInput").ap())
        boR_d.append(nc.dram_tensor(f"boR{l}", [1, DS], bf, kind="ExternalInput").ap())
    VSP = 4096
    WoutT_d = nc.dram_tensor("WoutT", [128, 2 * VSP], DT.float32r, kind="ExternalInput").ap()
    bout_d = nc.dram_tensor("bout", [1, VSP], DT.float32r, kind="ExternalInput").ap()
    onesr_d = nc.dram_tensor("onesr", [1, 128], DT.float32r, kind="ExternalInput").ap()

    logits_d = nc.dram_tensor("logits", [R, VSP], f32, kind="ExternalOutput").ap()
    ti_dbg_d = nc.dram_tensor("ti_dbg", [128, 2 * R], f32, kind="ExternalOutput").ap()

    # internal DRAM bounce buffers (for partition broadcasts)
    rc_b = nc.dram_tensor("rc_b", [NH, 512], f32).ap()
    th_b = nc.dram_tensor("th_b", [2, 1], f32).ap()

    with tile.TileContext(nc) as tc:
        with tc.tile_pool(name="state", bufs=1) as st, \
             tc.tile_pool(name="wk", bufs=1) as wk, \
             tc.tile_pool(name="sc", bufs=3) as sc, \
             tc.tile_pool(name="pr", bufs=6) as pr, \
             tc.tile_pool(name="psB", bufs=2, space="PSUM") as psB, \
             tc.tile_pool(name="psS", bufs=4, space="PSUM") as psS:

            # ---------------- persistent SBUF ----------------
            ident = st.tile([128, 128], f32)
            make_identity(nc, ident)
            identb = st.tile([128, 128], bf)
            make_identity(nc, identb)
            ones512 = st.tile([1, 512], bf)
            nc.vector.memset(ones512[:], 1.0)
            ones_r = st.tile([1, 128], bf)       # ones row, lhsT for bias mms
            nc.vector.memset(ones_r[:], 1.0)
            ones_rf = st.tile([1, 128], DT.float32r)
            nc.sync.dma_start(out=ones_rf[:], in_=onesr_d[:])
            ones_c = st.tile([128, 1], f32)      # ones col, partition reduce
            nc.vector.memset(ones_c[:], 1.0)

            x0T = st.tile([128, 2 * R], bf)      # tok transposed, 2 dm-chunks
            k1T = st.tile([128, R], bf)
            k2T = st.tile([128, R], bf)
            k2Tb = st.tile([128, R], bf)
            v1p = st.tile([128, 8 * 132], bf)    # [kpos, rt*(4 heads*(32v+1))]
            v2p = st.tile([128, 8 * 132], bf)
            v2pb = st.tile([128, 8 * 132], bf)
            hT = st.tile([128, R], bf)
            svT = st.tile([128, R], bf)          # stored pre-decayed
            ovT = st.tile([128, 2 * R], bf)      # 2 dm-chunks, pre-decayed
            thr_s = st.tile([128, 1], f32)
            thr_o = st.tile([128, 1], f32)
            os1 = st.tile([128, T * 2 * R], bf)  # layer-1 spikes, all steps
            tiT = st.tile([128, 2 * R], f32)
            attn = st.tile([128, R], bf)
            q_sb = st.tile([128, R], bf)
            keep = st.tile([128, R], f32)
            os2 = st.tile([128, 2 * R], bf)
            scnt = st.tile([128, 8], f32)        # spike rowsums per half/block
            thrw = st.tile([1, 4], f32)          # small thr scratch

            # weights
            ATs, WqTs, WkTs, WvTs, WoTs, CTs = [], [], [], [], [], []
            bqs, bks, bvs, bos, boRs = [], [], [], [], []
            for l in range(L):
                ATs.append(wk.tile([DS, DS], bf, tag=f"AT{l}", name=f"ATs{l}"))
                WqTs.append(wk.tile([DS, DS], bf, tag=f"Wq{l}", name=f"WqTs{l}"))
                WkTs.append(wk.tile([128, 256], bf, tag=f"Wk{l}", name=f"WkTs{l}"))
                WvTs.append(wk.tile([128, 256], bf, tag=f"Wv{l}", name=f"WvTs{l}"))
                WoTs.append(wk.tile([DS, DS], bf, tag=f"Wo{l}", name=f"WoTs{l}"))
                CTs.append(wk.tile([DS, DM], bf, tag=f"CT{l}", name=f"CTs{l}"))
                bqs.append(wk.tile([DS, 1], f32, tag=f"bq{l}", name=f"bqs{l}"))
                bks.append(wk.tile([DS, 1], f32, tag=f"bk{l}", name=f"bks{l}"))
                bvs.append(wk.tile([1, DS], bf, tag=f"bv{l}", name=f"bvs{l}"))
                bos.append(wk.tile([DS, 1], f32, tag=f"bo{l}", name=f"bos{l}"))
                boRs.append(wk.tile([1, DS], bf, tag=f"boR{l}", name=f"boRs{l}"))
                nc.sync.dma_start(out=boRs[l][:], in_=boR_d[l][:])
                nc.sync.dma_start(out=ATs[l][:], in_=AT_d[l][:])
                nc.sync.dma_start(out=WqTs[l][:], in_=WqT_d[l][:])
                nc.sync.dma_start(out=WkTs[l][:], in_=WkT_d[l][:])
                nc.sync.dma_start(out=WvTs[l][:], in_=WvT_d[l][:])
                nc.sync.dma_start(out=WoTs[l][:], in_=WoT_d[l][:])
                nc.sync.dma_start(out=CTs[l][:], in_=CT_d[l][:])
                nc.sync.dma_start(out=bqs[l][:], in_=bq_d[l][:])
                nc.sync.dma_start(out=bks[l][:], in_=bk_d[l][:])
                nc.sync.dma_start(out=bvs[l][:], in_=bv_d[l][:])
                nc.sync.dma_start(out=bos[l][:], in_=bo_d[l][:])
            Wout_sb = wk.tile([128, 2 * VSP], DT.float32r, tag="wout")
            nc.sync.dma_start(out=Wout_sb[:], in_=WoutT_d[:])
            bout_sb = wk.tile([1, VSP], DT.float32r, tag="bout")
            nc.sync.dma_start(out=bout_sb[:], in_=bout_d[:])

            # ---------------- embedding gather + transpose ----------------
            for rt in range(8):
                idx_sb = sc.tile([128, 1], i32, tag="idx")
                nc.sync.dma_start(out=idx_sb[:], in_=idx_d[128 * rt:128 * (rt + 1), :])
                tok_sb = sc.tile([128, DM], f32, tag="tok")
                nc.gpsimd.indirect_dma_start(
                    out=tok_sb[:], out_offset=None, in_=emb_d[:],
                    in_offset=bass.IndirectOffsetOnAxis(ap=idx_sb[:, :1], axis=0))
                for mc in range(2):
                    ps_t = psS.tile([128, 128], f32, tag="pa")
                    nc.tensor.transpose(out=ps_t[:], in_=tok_sb[:, 128 * mc:128 * (mc + 1)],
                                        identity=ident[:])
                    nc.vector.tensor_copy(
                        out=x0T[:, R * mc + 128 * rt: R * mc + 128 * (rt + 1)],
                        in_=ps_t[:])

            # ---------------- init state ----------------
            nc.gpsimd.memset(hT[:], 0.0)
            nc.gpsimd.memset(svT[:], 0.0)
            nc.gpsimd.memset(ovT[:], 0.0)
            nc.gpsimd.memset(tiT[:], 0.0)
            nc.vector.memset(thr_s[:], BASE_THR)
            nc.vector.memset(thr_o[:], BASE_THR)
            # ones columns of v_pos tiles
            for vp in (v1p, v2p, v2pb):
                vv = vp[:].rearrange("p (r h c) -> p r h c", r=8, h=NH, c=33)
                nc.gpsimd.memset(vv[:, :, :, 32:33], 1.0)

            def kv_for(l, xT_ap_fn, kdst, vdst):
                """xT_ap_fn(kc) -> [128, R] feature-major input chunk."""
                for nh in range(2):
                    ps_k = psS.tile([128, 512], f32, tag="pa")
                    for kc in range(2):
                        nc.tensor.matmul(ps_k[:],
                                         lhsT=WkTs[l][:, 128 * kc:128 * (kc + 1)],
                                         rhs=xT_ap_fn(kc)[:, 512 * nh:512 * (nh + 1)],
                                         start=(kc == 0), stop=(kc == 1))
                    nc.vector.tensor_scalar(out=kdst[:, 512 * nh:512 * (nh + 1)],
                                            in0=ps_k[:], scalar1=bks[l][:, :1],
                                            scalar2=None, op0=OP.add)
                vv = vdst[:].rearrange("p (r h c) -> p r h c", r=8, h=NH, c=33)
                for rt in range(8):
                    ps_v = psS.tile([128, 128], f32, tag="pa")
                    for kc in range(2):
                        nc.tensor.matmul(ps_v[:],
                                         lhsT=xT_ap_fn(kc)[:, 128 * rt:128 * (rt + 1)],
                                         rhs=WvTs[l][:, 128 * kc:128 * (kc + 1)],
                                         start=(kc == 0), stop=False)
                    nc.tensor.matmul(ps_v[:], lhsT=ones_r[:], rhs=bvs[l][:],
                                     start=False, stop=True)
                    src = ps_v[:].rearrange("p (h c) -> p h c", h=NH, c=32)
                    nc.vector.tensor_copy(out=vv[:, rt, :, 0:32], in_=src)

            def lif(ps_upd, sv_ap, thr_ap, spike_dst, scnt_col, kslice):
                """Adaptive-LIF on a [128, 512] half-block.
                ps_upd already holds upd + sv(pre-decayed) (+ bias)."""
                nc.vector.tensor_scalar(out=spike_dst, in0=ps_upd[:],
                                        scalar1=thr_ap, scalar2=None, op0=OP.is_ge)
                nc.vector.tensor_scalar(out=keep[:, kslice], in0=ps_upd[:],
                                        scalar1=thr_ap, scalar2=DECAY,
                                        op0=OP.is_lt, op1=OP.mult)
                nc.vector.tensor_tensor(out=sv_ap, in0=ps_upd[:], in1=keep[:, kslice],
                                        op=OP.mult)
                nc.vector.tensor_reduce(scnt[:, scnt_col:scnt_col + 1], spike_dst,
                                        axis=mybir.AxisListType.X, op=OP.add)

            def thr_update(thr_tile, cols, denom, th_row):
                """thr += ADAPT*(mean-TARGET); thr = max(thr, 0.5); rebroadcast."""
                ps_tot = psS.tile([1, 1], f32, tag="pa")
                for i, c in enumerate(cols):
                    nc.tensor.matmul(ps_tot[:], lhsT=ones_c[:],
                                     rhs=scnt[:, c:c + 1],
                                     start=(i == 0), stop=(i == len(cols) - 1))
                d = thrw[:, th_row:th_row + 1]
                nc.vector.tensor_scalar(out=d, in0=ps_tot[:],
                                        scalar1=ADAPT / denom,
                                        scalar2=-ADAPT * TARGET,
                                        op0=OP.mult, op1=OP.add)
                nc.vector.tensor_tensor(out=d, in0=d, in1=thr_tile[0:1, :], op=OP.add)
                nc.vector.tensor_scalar(out=d, in0=d, scalar1=0.5, scalar2=None,
                                        op0=OP.max)
                nc.sync.dma_start(out=th_b[th_row:th_row + 1, :], in_=d)
                nc.sync.dma_start(out=thr_tile[:],
                                  in_=th_b[th_row:th_row + 1, :].to_broadcast((128, 1)))

            # ---------------- the recurrence ----------------
            kv_for(0, lambda kc: x0T[:, R * kc:R * (kc + 1)], k1T, v1p)

            for l in range(L):
                for t in range(T):
                    if l == 0:
                        k_sb, v_sb = k1T, v1p
                    else:
                        k_sb = k2T if t % 2 == 0 else k2Tb
                        v_sb = v2p if t % 2 == 0 else v2pb
                        xfn = lambda kc, t=t: os1[:, 2 * R * t + R * kc:2 * R * t + R * (kc + 1)]
                        kv_for(1, xfn, k_sb, v_sb)

                    for qh in range(2):
                        Q = slice(512 * qh, 512 * (qh + 1))

                        # qT = Wq @ hT + bq  (bias applied on ScalarE)
                        ps_q = psS.tile([128, 512], f32, tag="pa")
                        nc.tensor.matmul(ps_q[:], lhsT=WqTs[l][:], rhs=hT[:, Q],
                                         start=True, stop=True)
                        nc.scalar.add(out=q_sb[:, Q], in_=ps_q[:], add=bqs[l][:, :1])

                        # attention: scoresT -> exp -> att (+sums via ones col)
                        pa = [psS.tile([128, 512], f32, tag="pa", name=f"pa{p}")
                              for p in range(2)]
                        for kt in range(8):
                            for p in range(2):
                                ps_sc = psB.tile([128, R], f32, tag="big")
                                for j in range(2):
                                    h = 2 * p + j
                                    nc.tensor.matmul(
                                        ps_sc[:, 512 * j:512 * (j + 1)],
                                        lhsT=k_sb[32 * h:32 * (h + 1), 128 * kt:128 * (kt + 1)],
                                        rhs=q_sb[32 * h:32 * (h + 1), Q],
                                        start=True, stop=True,
                                        tile_position=(32 * h, 0))
                                probs = pr.tile([128, R], bf, tag="probs")
                                nc.scalar.activation(probs[:], ps_sc[:], AF.Exp,
                                                     bias=0.0, scale=SCALE)
                                for j in range(2):
                                    h = 2 * p + j
                                    nc.tensor.matmul(
                                        pa[p][64 * j:64 * j + 33, :],
                                        lhsT=v_sb[:, 132 * kt + 33 * h:132 * kt + 33 * (h + 1)],
                                        rhs=probs[:, 512 * j:512 * (j + 1)],
                                        start=(kt == 0), stop=(kt == 7),
                                        tile_position=(0, 64 * j))
                        srows = sc.tile([97, 512], f32, tag="srows")
                        for h in range(NH):
                            p, j = h // 2, h % 2
                            nc.vector.tensor_copy(out=srows[32 * h:32 * h + 1, :],
                                                  in_=pa[p][64 * j + 32:64 * j + 33, :])
                        recip = sc.tile([97, 512], f32, tag="recip")
                        nc.vector.reciprocal_approx_fast(out=recip[:], in_=srows[:])
                        for h in range(NH):
                            nc.sync.dma_start(out=rc_b[h:h + 1, :],
                                              in_=recip[32 * h:32 * h + 1, :])
                        rb = sc.tile([128, 512], f32, tag="rb")
                        for h in range(NH):
                            nc.sync.dma_start(
                                out=rb[32 * h:32 * (h + 1), :],
                                in_=rc_b[h:h + 1, :].to_broadcast((32, 512)))
                        for h in range(NH):
                            p, j = h // 2, h % 2
                            nc.vector.tensor_tensor(
                                out=attn[32 * h:32 * (h + 1), Q],
                                in0=pa[p][64 * j:64 * j + 32, :],
                                in1=rb[32 * h:32 * (h + 1), :], op=OP.mult)

                        # upd = A@hT + Wo@attn + sv + bo  (all accumulated in PSUM)
                        ps_u = psS.tile([128, 512], f32, tag="pa")
                        nc.tensor.matmul(ps_u[:], lhsT=ATs[l][:], rhs=hT[:, Q],
                                         start=True, stop=False)
                        nc.tensor.matmul(ps_u[:], lhsT=WoTs[l][:], rhs=attn[:, Q],
                                         start=False, stop=False)
                        nc.tensor.matmul(ps_u[:], lhsT=identb[:], rhs=svT[:, Q],
                                         start=False, stop=False)
                        nc.tensor.matmul(ps_u[:], lhsT=boRs[l][:], rhs=ones512[:],
                                         start=False, stop=True)
                        lif(ps_u, svT[:, Q], thr_s[:, :1], hT[:, Q], 4 * qh + 0, Q)

                        # out_pot = C @ h2T + ov ; second LIF
                        for mc in range(2):
                            ps_o = psS.tile([128, 512], f32, tag="pa")
                            nc.tensor.matmul(ps_o[:], lhsT=CTs[l][:, 128 * mc:128 * (mc + 1)],
                                             rhs=hT[:, Q], start=True, stop=False)
                            nc.tensor.matmul(ps_o[:], lhsT=identb[:],
                                             rhs=ovT[:, R * mc + 512 * qh:R * mc + 512 * (qh + 1)],
                                             start=False, stop=True)
                            if l == 0:
                                sdst = os1[:, 2 * R * t + R * mc + 512 * qh:
                                           2 * R * t + R * mc + 512 * (qh + 1)]
                            else:
                                sdst = os2[:, R * mc + 512 * qh:R * mc + 512 * (qh + 1)]
                            lif(ps_o, ovT[:, R * mc + 512 * qh:R * mc + 512 * (qh + 1)],
                                thr_o[:, :1], sdst, 4 * qh + 1 + mc, Q)
                            if l == 1:
                                nc.vector.tensor_tensor(
                                    out=tiT[:, R * mc + 512 * qh:R * mc + 512 * (qh + 1)],
                                    in0=tiT[:, R * mc + 512 * qh:R * mc + 512 * (qh + 1)],
                                    in1=sdst, op=OP.add)

                    thr_update(thr_s, [0, 4], float(R * DS), 0)
                    thr_update(thr_o, [1, 2, 5, 6], float(R * DM), 1)

                if l == 0:
                    # reset state for layer 2
                    nc.gpsimd.memset(hT[:], 0.0)
                    nc.gpsimd.memset(svT[:], 0.0)
                    nc.gpsimd.memset(ovT[:], 0.0)
                    nc.vector.memset(thr_s[:], BASE_THR)
                    nc.vector.memset(thr_o[:], BASE_THR)

            # time integration (mean over T)
            tiP = st.tile([128, 2 * R], DT.float32r)
            nc.vector.tensor_scalar(out=tiP[:], in0=tiT[:], scalar1=1.0 / T,
                                    scalar2=None, op0=OP.mult)
            nc.sync.dma_start(out=ti_dbg_d[:], in_=tiP[:].bitcast(DT.float32))

            # ---------------- output projection (fp32r) ----------------
            ti_r = tiP[:]
            w_r = Wout_sb[:]
            b_r = bout_sb[:]
            ones_rr = ones_rf[:]
            nchunk = [(i * 512, 512) for i in range(VSP // 512)]
            for pt in range(8):
                for (c0, cw) in nchunk:
                    ps_l = psS.tile([128, 512], f32, tag="pa")
                    nc.tensor.matmul(ps_l[:, :cw], lhsT=ones_rr[:, :128],
                                     rhs=b_r[:, c0:c0 + cw], start=True, stop=False)
                    for kc in range(2):
                        nc.tensor.matmul(
                            ps_l[:, :cw],
                            lhsT=ti_r[:, R * kc + 128 * pt:R * kc + 128 * (pt + 1)],
                            rhs=w_r[:, VSP * kc + c0:VSP * kc + c0 + cw],
                            start=False, stop=(kc == 1))
                    o_sb = sc.tile([128, 512], f32, tag="osb")
                    if (pt + (c0 // 512)) % 2 == 0:
                        nc.vector.tensor_copy(out=o_sb[:, :cw], in_=ps_l[:, :cw])
                    else:
                        nc.scalar.copy(out=o_sb[:, :cw], in_=ps_l[:, :cw])
                    nc.sync.dma_start(out=logits_d[128 * pt:128 * (pt + 1), c0:c0 + cw],
                                      in_=o_sb[:, :cw])

    nc.compile()
    _CACHE["nc"] = nc
    return nc


def kernel(input_ids, emb, A, C, Wq, bq, Wkv, bkv, Wo, bo, Wout, bout):
    nc = _build()

    input_ids = np.asarray(input_ids)
    emb = np.ascontiguousarray(np.asarray(emb, dtype=np.float32))
    A = np.asarray(A, dtype=np.float32)
    C = np.asarray(C, dtype=np.float32)
    Wq = np.asarray(Wq, dtype=np.float32)
    bq = np.asarray(bq, dtype=np.float32)
    Wkv = np.asarray(Wkv, dtype=np.float32)
    bkv = np.asarray(bkv, dtype=np.float32)
    Wo = np.asarray(Wo, dtype=np.float32)
    bo = np.asarray(bo, dtype=np.float32)
    Wout = np.asarray(Wout, dtype=np.float32)
    bout = np.asarray(bout, dtype=np.float32)

    idx = np.ascontiguousarray(input_ids.reshape(R, 1).astype(np.int32))

    base = {"idx": idx, "emb": emb}
    for l in range(L):
        Wk = Wkv[l][:DS]      # [128, 256]
        Wv = Wkv[l][DS:]      # [128, 256]
        WkT = Wk.T            # [256, 128]
        WvT = Wv.T
        base[f"AT{l}"] = np.ascontiguousarray(A[l].T).astype(bf16)
        base[f"WqT{l}"] = np.ascontiguousarray(Wq[l].T).astype(bf16)
        base[f"WkT{l}"] = np.ascontiguousarray(
            np.concatenate([WkT[:128], WkT[128:]], axis=1)).astype(bf16)
        base[f"WvT{l}"] = np.ascontiguousarray(
            np.concatenate([WvT[:128], WvT[128:]], axis=1)).astype(bf16)
        base[f"WoT{l}"] = np.ascontiguousarray(Wo[l].T).astype(bf16)
        base[f"CT{l}"] = np.ascontiguousarray(C[l].T).astype(bf16)
        base[f"bq{l}"] = np.ascontiguousarray(bq[l].reshape(DS, 1))
        base[f"bk{l}"] = np.ascontiguousarray(bkv[l][:DS].reshape(DS, 1))
        base[f"bv{l}"] = np.ascontiguousarray(bkv[l][DS:].reshape(1, DS)).astype(bf16)
        base[f"bo{l}"] = np.ascontiguousarray(bo[l].reshape(DS, 1))
        base[f"boR{l}"] = np.ascontiguousarray(bo[l].reshape(1, DS)).astype(bf16)

    in_maps = []
    for c in range(NCORES):
        m = dict(base)
        VSP = 4096
        WoT_c = np.zeros((256, VSP), np.float32)
        WoT_c[:, :VS] = Wout[c * VS:(c + 1) * VS].T
        m["WoutT"] = np.ascontiguousarray(
            np.concatenate([WoT_c[:128], WoT_c[128:]], axis=1))
        bo_c = np.zeros((1, VSP), np.float32)
        bo_c[0, :VS] = bout[c * VS:(c + 1) * VS]
        m["bout"] = bo_c
        m["onesr"] = np.ones((1, 128), np.float32)
        in_maps.append(m)

    res = run_bass_kernel_spmd(nc, in_maps, list(range(NCORES)))
    kernel.last_results = res

    logits = np.concatenate([res.results[c]["logits"][:, :VS]
                             for c in range(NCORES)], axis=1)
    return logits.reshape(B, S, V)
